# revision 1
# baseline (speedup 1.0000x reference)
"""Trainium2 Bass kernel for nn_HalfConv_876173328516 (GNN message passing).

Strategy
--------
Host: sort edges by e_idx_u; core k owns u rows [k*6250, (k+1)*6250), so the 8
cores are fully independent (no collectives). Per-edge inputs are expanded and
transposed on the host into a [128, NE] u||v feature stream plus a packed
[48, NE/2] e_vals stream per core, with edges packed into 512-edge chunks that
(a) never split one u across chunks and (b) span < 64 u-slots.

Device (per core, per 512-edge chunk):
  L1   z1 = W1e.T @ e_chunk (K=16, two chunks run concurrently in disjoint
       PE row-groups) + W1uv.T @ x[:128]  (PSUM accumulate)
       h1 = relu(z1 + b1)                          (ACT, -> SBUF bf16)
  L2   per 128-edge group: z2T = h1_g.T @ W2       (transposed-output matmul)
       h2T = relu(z2T + b2)                        (DVE add + max -> [e, 64])
  SUM  one-hot[e, slot] = (iota == slot_col[e])    (one DVE is_equal, bf16)
       pT[64 feats, 64 slots] += h2T_g.T @ oh_g    (PSUM accumulate, 4 groups)
       flush pT -> xf[0:64, 64c:64c+64]            (DVE copy, bf16)
  f-MLP over all compact slot columns:  xf = [aggT ; uT], two matmuls + relus.

Host: out[u] = out_T[:, col_of_slot[u]].T per core.
"""

import numpy as np

U, V, E = 50000, 50000, 800000
NCORES = 8
U_PER = U // NCORES          # 6250
CHUNK = 512                  # edges per chunk
GP = 128                     # edges per matmul group
GROUPS = CHUNK // GP         # 4
WSLOTS = 64                  # slot window per chunk


# ---------------------------------------------------------------- host side

def _preprocess(u, v, e_vals, e_idx_v, e_idx_u):
    u = np.ascontiguousarray(np.asarray(u, np.float32))
    v = np.ascontiguousarray(np.asarray(v, np.float32))
    e_vals = np.ascontiguousarray(np.asarray(e_vals, np.float32))
    e_idx_u = np.asarray(e_idx_u).astype(np.int64)
    e_idx_v = np.asarray(e_idx_v).astype(np.int64)

    perm = np.argsort(e_idx_u, kind="stable")
    su = e_idx_u[perm]
    sv = e_idx_v[perm]
    se = e_vals[perm]

    core_lo = np.searchsorted(su, np.arange(NCORES) * U_PER, side="left")
    core_hi = np.searchsorted(su, (np.arange(NCORES) + 1) * U_PER, side="left")

    cores = []
    for k in range(NCORES):
        lo, hi = int(core_lo[k]), int(core_hi[k])
        su_l = (su[lo:hi] - k * U_PER).astype(np.int64)
        n = hi - lo
        starts, bases = [], []
        i = 0
        while i < n:
            base = int(su_l[i])
            j = min(i + CHUNK, n)
            j = min(j, int(np.searchsorted(su_l, base + WSLOTS, side="left")))
            if j < n:
                # step back to a u-boundary so no u straddles chunks
                j2 = int(np.searchsorted(su_l, su_l[j - 1], side="left"))
                if j2 > i and su_l[j - 1] == su_l[j]:
                    j = j2
            assert j > i, "u degree >= CHUNK unsupported"
            starts.append(i)
            bases.append(base)
            i = j
        starts.append(n)
        nchunks = len(bases)

        col_of_slot = np.full(U_PER, -1, np.int64)
        for c in range(nchunks):
            s0, s1 = starts[c], starts[c + 1]
            slots = np.unique(su_l[s0:s1])
            assert slots.max() - bases[c] < WSLOTS
            col_of_slot[slots] = WSLOTS * c + (slots - bases[c])
        uncovered = np.flatnonzero(col_of_slot < 0)
        cores.append(dict(lo=lo, hi=hi, su_l=su_l, starts=starts, bases=bases,
                          nchunks=nchunks, col_of_slot=col_of_slot,
                          uncovered=uncovered))

    need = max(c["nchunks"] + (len(c["uncovered"]) + WSLOTS - 1) // WSLOTS + 1
               for c in cores)
    B = need + (need % 2)            # blocks of 2 chunks
    NE = B * CHUNK
    C = B * WSLOTS

    per_core = []
    for k in range(NCORES):
        ci = cores[k]
        lo, hi = ci["lo"], ci["hi"]
        su_l, starts, bases = ci["su_l"], ci["starts"], ci["bases"]
        nchunks = ci["nchunks"]

        col_of_slot = ci["col_of_slot"].copy()
        unc = ci["uncovered"]
        if len(unc):
            cols = WSLOTS * nchunks + np.arange(len(unc))
            assert cols.max() < C
            col_of_slot[unc] = cols
        assert (col_of_slot >= 0).all()

        x_T = np.zeros((128, NE), np.float32)      # [uT ; vT]
        e2_T = np.zeros((48, NE // 2), np.float32)  # rows 0:16 even, 32:48 odd
        slot_cols = np.full((GP, GROUPS * B), -1.0, np.float32)
        n = hi - lo
        if n:
            x_src = np.empty((128, n), np.float32)
            x_src[0:64] = u[su[lo:hi]].T
            x_src[64:128] = v[sv[lo:hi]].T
            e_src = se[lo:hi].T                     # [16, n]
        for c in range(nchunks):
            s0, s1 = starts[c], starts[c + 1]
            m = s1 - s0
            x_T[:, c * CHUNK:c * CHUNK + m] = x_src[:, s0:s1]
            r0 = 0 if c % 2 == 0 else 32
            col0 = (c // 2) * CHUNK
            e2_T[r0:r0 + 16, col0:col0 + m] = e_src[:, s0:s1]
            full = np.full(CHUNK, -1.0, np.float32)
            full[:m] = (su_l[s0:s1] - bases[c]).astype(np.float32)
            slot_cols[:, GROUPS * c:GROUPS * (c + 1)] = \
                full.reshape(GROUPS, GP).T

        u_T_compact = np.zeros((64, C), np.float32)
        u_T_compact[:, col_of_slot] = u[k * U_PER:(k + 1) * U_PER].T

        per_core.append(dict(x_T=x_T, e2_T=e2_T, slot_cols=slot_cols,
                             u_T_compact=u_T_compact,
                             col_of_slot=col_of_slot))
    return per_core, B, NE, C


# ---------------------------------------------------------------- device side

def _build_program(B, NE, C, io_dtype_np):
    import concourse.bacc as bacc
    import concourse.mybir as mybir
    import concourse.tile as tile

    FB = (C + 511) // 512               # f-MLP chunks
    md = mybir.dt.from_np(np.dtype(io_dtype_np))
    f32 = mybir.dt.float32
    Relu = mybir.ActivationFunctionType.Relu
    Alu = mybir.AluOpType

    nc = bacc.Bacc("TRN2", target_bir_lowering=False, debug=False,
                   num_devices=NCORES)

    # I/O
    x_T = nc.dram_tensor("x_T", [128, NE], md, kind="ExternalInput")
    e2_T = nc.dram_tensor("e2_T", [48, NE // 2], md, kind="ExternalInput")
    slot_cols = nc.dram_tensor("slot_cols", [GP, GROUPS * B], md,
                               kind="ExternalInput")
    u_Tc = nc.dram_tensor("u_Tc", [64, C], md, kind="ExternalInput")
    w1a = nc.dram_tensor("w1a", [128, 128], md, kind="ExternalInput")
    w1e2 = nc.dram_tensor("w1e2", [48, 128], md, kind="ExternalInput")
    w2 = nc.dram_tensor("w2", [128, 64], md, kind="ExternalInput")
    fw1 = nc.dram_tensor("fw1", [128, 128], md, kind="ExternalInput")
    fw2 = nc.dram_tensor("fw2", [128, 64], md, kind="ExternalInput")
    b1 = nc.dram_tensor("b1", [128, 1], f32, kind="ExternalInput")
    b2m = nc.dram_tensor("b2m", [GP, 64 * GROUPS], f32, kind="ExternalInput")
    fb1 = nc.dram_tensor("fb1", [128, 1], f32, kind="ExternalInput")
    fb2 = nc.dram_tensor("fb2", [64, 1], f32, kind="ExternalInput")
    iota4 = nc.dram_tensor("iota4", [GP, 64 * GROUPS], md,
                           kind="ExternalInput")
    out_T = nc.dram_tensor("out_T", [64, C], f32, kind="ExternalOutput")

    with tile.TileContext(nc) as tc:
        with (
            tc.tile_pool(name="consts", bufs=1) as cp,
            tc.tile_pool(name="xf", bufs=1) as xfp,
            tc.tile_pool(name="xin", bufs=3) as xp,
            tc.tile_pool(name="work", bufs=3) as wp,
            tc.tile_pool(name="ps1", bufs=4, space="PSUM") as p1,
            tc.tile_pool(name="ps2", bufs=1, space="PSUM") as p2,
            tc.tile_pool(name="ps3", bufs=2, space="PSUM") as p3,
            tc.tile_pool(name="psf", bufs=1, space="PSUM") as pf,
        ):
            # resident constants
            w1a_s = cp.tile([128, 128], md)
            w1e2_s = cp.tile([48, 128], md)
            w2_s = cp.tile([128, 64], md)
            fw1_s = cp.tile([128, 128], md)
            fw2_s = cp.tile([128, 64], md)
            b1_s = cp.tile([128, 1], f32)
            b2m_s = cp.tile([GP, 64 * GROUPS], f32)
            fb1_s = cp.tile([128, 1], f32)
            fb2_s = cp.tile([64, 1], f32)
            iota_s = cp.tile([GP, 64 * GROUPS], md)
            slot_s = cp.tile([GP, GROUPS * B], md)
            for dst, src in [(w1a_s, w1a), (w1e2_s, w1e2), (w2_s, w2),
                             (fw1_s, fw1), (fw2_s, fw2), (b1_s, b1),
                             (b2m_s, b2m), (fb1_s, fb1), (fb2_s, fb2),
                             (iota_s, iota4), (slot_s, slot_cols)]:
                nc.sync.dma_start(dst[:], src[:])

            # xf: rows 0:64 aggT (flushed per chunk), rows 64:128 uT
            xf = xfp.tile([128, C], md)
            nc.sync.dma_start(xf[64:128, :], u_Tc[:])

            # f-MLP chunk emitter (interleaved into the main loop)
            f_done = [0]

            def emit_f(fc):
                w = min(512, C - 512 * fc)
                fsl = slice(512 * fc, 512 * fc + w)
                zf1 = pf.tile([128, 512], f32, tag="zf")
                nc.tensor.matmul(zf1[:, :w], lhsT=fw1_s[:], rhs=xf[:, fsl],
                                 start=True, stop=True)
                hf = wp.tile([128, 512], md, tag="hf")
                nc.scalar.activation(hf[:, :w], zf1[:, :w], Relu,
                                     bias=fb1_s[:])
                zf2 = pf.tile([64, 512], f32, tag="zf")
                nc.tensor.matmul(zf2[:, :w], lhsT=fw2_s[:], rhs=hf[:, :w],
                                 start=True, stop=True)
                ot = wp.tile([64, 512], f32, tag="ot")
                nc.scalar.activation(ot[:, :w], zf2[:, :w], Relu,
                                     bias=fb2_s[:])
                nc.sync.dma_start(out_T[:, fsl], ot[:, :w])
                f_done[0] = fc + 1

            for m in range(B // 2):                 # block = 2 chunks
                if m % 4 == 0:                      # 4-block x1 superbatch
                    nb = min(4, B // 2 - m)
                    x1 = xp.tile([128, 4 * 2 * CHUNK], md, tag="x1")
                    nc.sync.dma_start(
                        x1[:, :nb * 2 * CHUNK],
                        x_T[:, m * 2 * CHUNK:(m + nb) * 2 * CHUNK])
                x1off = (m % 4) * 2 * CHUNK
                if m % 2 == 0:                      # 2-block e2 batch
                    nb = min(2, B // 2 - m)
                    e2 = xp.tile([48, 2 * CHUNK], md, tag="e2")
                    nc.sync.dma_start(
                        e2[:, :nb * CHUNK],
                        e2_T[:, m * CHUNK:(m + nb) * CHUNK])
                e2off = (m % 2) * CHUNK

                # all four N=512 matmuls back-to-back (drain/fill overlap)
                z1s = []
                for q in range(2):
                    z1 = p1.tile([128, CHUNK], f32, tag="z1")
                    z1s.append(z1)
                    r0 = 32 * q
                    nc.tensor.matmul(z1[:], lhsT=w1e2_s[r0:r0 + 16, :],
                                     rhs=e2[r0:r0 + 16, e2off:e2off + CHUNK],
                                     start=True, stop=False,
                                     tile_position=(r0, 0),
                                     skip_group_check=True)
                for q in range(2):
                    nc.tensor.matmul(z1s[q][:], lhsT=w1a_s[:],
                                     rhs=x1[:, x1off + q * CHUNK:
                                            x1off + (q + 1) * CHUNK],
                                     start=False, stop=True,
                                     skip_group_check=True)
                pT = p3.tile([64, 2 * WSLOTS], f32, tag="pT")
                for q in range(2):                  # per-chunk tail
                    c = 2 * m + q
                    h1 = wp.tile([128, CHUNK], md, tag="h1")
                    nc.scalar.activation(h1[:], z1s[q][:], Relu, bias=b1_s[:])

                    z2 = p2.tile([GP, 64 * GROUPS], f32, tag="z2")
                    for g in range(GROUPS):
                        nc.tensor.matmul(z2[:, 64 * g:64 * (g + 1)],
                                         lhsT=h1[:, GP * g:GP * (g + 1)],
                                         rhs=w2_s[:], start=True, stop=True)
                    h2T = wp.tile([GP, 64 * GROUPS], md, tag="h2T")
                    nc.vector.tensor_tensor(h2T[:], z2[:], b2m_s[:],
                                            op=Alu.add)
                    nc.vector.tensor_scalar_max(h2T[:], h2T[:], 0.0)

                    oh = wp.tile([GP, 64 * GROUPS], md, tag="oh")
                    nc.vector.tensor_tensor(
                        oh[:].rearrange("p (g s) -> p g s", g=GROUPS),
                        iota_s[:].rearrange("p (g s) -> p g s", g=GROUPS),
                        slot_s[:, GROUPS * c:GROUPS * (c + 1)][:, :, None]
                            .to_broadcast([GP, GROUPS, WSLOTS]),
                        op=Alu.is_equal)

                    for g in range(GROUPS):
                        nc.tensor.matmul(pT[:, 64 * q:64 * (q + 1)],
                                         lhsT=h2T[:, 64 * g:64 * (g + 1)],
                                         rhs=oh[:, 64 * g:64 * (g + 1)],
                                         start=(g == 0),
                                         stop=(g == GROUPS - 1))
                nc.vector.tensor_copy(
                    xf[0:64, 2 * WSLOTS * m:2 * WSLOTS * (m + 1)], pT[:])
                while (f_done[0] + 1) * 512 <= (m + 1) * 2 * WSLOTS:
                    emit_f(f_done[0])

            for fc in range(f_done[0], FB):
                emit_f(fc)

    nc.compile()
    return nc


def _make_in_maps(per_core, consts, io_dtype_np):
    in_maps = []
    for pc in per_core:
        m = dict(
            x_T=pc["x_T"].astype(io_dtype_np),
            e2_T=pc["e2_T"].astype(io_dtype_np),
            slot_cols=pc["slot_cols"].astype(io_dtype_np),
            u_Tc=pc["u_T_compact"].astype(io_dtype_np),
            **{k: v for k, v in consts.items()},
        )
        in_maps.append(m)
    return in_maps


def _make_consts(g_w1, g_b1, g_w2, g_b2, f_w1, f_b1, f_w2, f_b2, io_dtype_np):
    dt = io_dtype_np
    g_w1 = np.asarray(g_w1, np.float32)
    w1e2 = np.zeros((48, 128), np.float32)
    w1e2[0:16] = g_w1[128:144]
    w1e2[32:48] = g_w1[128:144]
    # f-MLP input is [aggT ; uT] (agg rows first), so permute f_w1 rows
    f_w1 = np.asarray(f_w1, np.float32)
    f_w1p = np.concatenate([f_w1[64:128], f_w1[0:64]], axis=0)
    return dict(
        w1a=np.ascontiguousarray(g_w1[0:128]).astype(dt),
        w1e2=w1e2.astype(dt),
        w2=np.asarray(g_w2, np.float32).astype(dt),
        fw1=np.ascontiguousarray(f_w1p).astype(dt),
        fw2=np.asarray(f_w2, np.float32).astype(dt),
        b1=np.asarray(g_b1, np.float32).reshape(128, 1),
        b2m=np.ascontiguousarray(
            np.tile(np.asarray(g_b2, np.float32)[None, :], (GP, GROUPS))),
        fb1=np.asarray(f_b1, np.float32).reshape(128, 1),
        fb2=np.asarray(f_b2, np.float32).reshape(64, 1),
        iota4=np.ascontiguousarray(
            np.tile(np.arange(WSLOTS, dtype=np.float32)[None, :],
                    (GP, GROUPS))).astype(dt),
    )


_last_run_info = {}


def kernel(u, v, e_vals, e_idx_v, e_idx_u, g_w1, g_b1, g_w2, g_b2,
           f_w1, f_b1, f_w2, f_b2, _trace=False):
    import ml_dtypes
    from concourse import bass_utils

    io_dtype_np = ml_dtypes.bfloat16

    per_core, B, NE, C = _preprocess(u, v, e_vals, e_idx_v, e_idx_u)
    consts = _make_consts(g_w1, g_b1, g_w2, g_b2, f_w1, f_b1, f_w2, f_b2,
                          io_dtype_np)
    nc = _build_program(B, NE, C, io_dtype_np)
    in_maps = _make_in_maps(per_core, consts, io_dtype_np)

    res = bass_utils.run_bass_kernel_spmd(
        nc, in_maps, core_ids=list(range(NCORES)), trace=_trace)
    _last_run_info.clear()
    _last_run_info.update(B=B, NE=NE, C=C, res=res)

    out = np.zeros((U, 64), np.float32)
    for k in range(NCORES):
        out_T = res.results[k]["out_T"]
        cols = per_core[k]["col_of_slot"]
        out[k * U_PER:(k + 1) * U_PER] = out_T[:, cols].T
    return out



# revision 2
# speedup vs baseline: 1.0221x; 1.0221x over previous
"""Trainium2 Bass kernel for nn_HalfConv_876173328516 (GNN message passing).

Strategy (v2)
-------------
Host: sort edges by e_idx_u; core k owns u rows [k*6250, (k+1)*6250) so the 8
cores are fully independent. Edges are packed into 512-edge chunks whose u
values span < 48 slots. Per chunk the host emits:
  - a [128, 512] moving stream  x = [v_e(64) ; e_vals_e(16) ; onehot48_e(48)]
  - a [128, 128] stationary     lt = [W1v(64) ; W1e(16) ; Au_window(48)]
where Au = u @ g_w1[:64] (the per-u L1 contribution, precomputed on host), so
ONE K=128 N=512 matmul computes the full g-MLP layer-1 preactivation:
  z1[:, e] = W1v.T v_e + W1e.T e_e + Au[slot(e)].

Device (per core, per 512-edge chunk):
  L1   z1 = lt_c.T @ x_c                  (one N=512 matmul)
       h1 = relu(z1 + b1)                 (ACT -> SBUF bf16)
  L2   per 128-edge group: z2T = h1_g.T @ W2   (4x N=64 matmuls)
       h2T = max(z2T, -b2)                (DVE; relu(z+b) = max(z,-b)+b,
                                           the +deg*b2 lands in the flush)
  SUM  oh[e, slot] one-hot streamed from host (bf16)
       pT[64 feats, 48 slots] += h2T_g.T @ oh_g  (4x N=48, PSUM accumulate)
       xf[0:64, cols] = pT + b2*deg       (DVE add flush, bf16)
  f-MLP over compact slot columns: xf = [aggT ; uT], two matmuls + relus.

L1 for block m+1 is issued ahead of block m's tails so the PE never starves.

Host: out[u] = out_T[:, col_of_slot[u]].T per core.
"""

import numpy as np

U, V, E = 50000, 50000, 800000
NCORES = 8
U_PER = U // NCORES          # 6250
CHUNK = 512                  # edges per chunk
GP = 128                     # edges per matmul group
GROUPS = CHUNK // GP         # 4
WS = 48                      # slot window per chunk
F_DIM, G_DIM, H_DIM = 64, 64, 16


# ---------------------------------------------------------------- host side

def _preprocess(u, v, e_vals, e_idx_v, e_idx_u, g_w1, io_dtype_np):
    u = np.ascontiguousarray(np.asarray(u, np.float32))
    v = np.ascontiguousarray(np.asarray(v, np.float32))
    e_vals = np.ascontiguousarray(np.asarray(e_vals, np.float32))
    e_idx_u = np.asarray(e_idx_u).astype(np.int64)
    e_idx_v = np.asarray(e_idx_v).astype(np.int64)
    g_w1 = np.asarray(g_w1, np.float32)

    perm = np.argsort(e_idx_u, kind="stable")
    su = e_idx_u[perm]
    sv = e_idx_v[perm]
    se = e_vals[perm]

    core_lo = np.searchsorted(su, np.arange(NCORES) * U_PER, side="left")
    core_hi = np.searchsorted(su, (np.arange(NCORES) + 1) * U_PER, side="left")

    cores = []
    for k in range(NCORES):
        lo, hi = int(core_lo[k]), int(core_hi[k])
        su_l = (su[lo:hi] - k * U_PER).astype(np.int64)
        n = hi - lo
        starts, bases = [], []
        i = 0
        while i < n:
            base = int(su_l[i])
            j = min(i + CHUNK, n)
            j = min(j, int(np.searchsorted(su_l, base + WS, side="left")))
            if j < n:
                # step back to a u-boundary so no u straddles chunks
                j2 = int(np.searchsorted(su_l, su_l[j - 1], side="left"))
                if j2 > i and su_l[j - 1] == su_l[j]:
                    j = j2
            assert j > i, "u degree >= CHUNK unsupported"
            starts.append(i)
            bases.append(base)
            i = j
        starts.append(n)
        nchunks = len(bases)

        col_of_slot = np.full(U_PER, -1, np.int64)
        for c in range(nchunks):
            s0, s1 = starts[c], starts[c + 1]
            slots = np.unique(su_l[s0:s1])
            assert slots.max() - bases[c] < WS
            col_of_slot[slots] = WS * c + (slots - bases[c])
        uncovered = np.flatnonzero(col_of_slot < 0)
        cores.append(dict(lo=lo, hi=hi, su_l=su_l, sv=sv[lo:hi],
                          se=se[lo:hi], starts=starts, bases=bases,
                          nchunks=nchunks, col_of_slot=col_of_slot,
                          uncovered=uncovered))

    need = max(c["nchunks"] + (len(c["uncovered"]) + WS - 1) // WS + 1
               for c in cores)
    B = need + (need % 2)            # chunks, in blocks of 2
    NE = B * CHUNK
    C = B * WS

    W1v = g_w1[F_DIM:F_DIM + G_DIM]              # [64, 128]
    W1e = g_w1[F_DIM + G_DIM:]                   # [16, 128]

    per_core = []
    for k in range(NCORES):
        ci = cores[k]
        su_l, starts, bases = ci["su_l"], ci["starts"], ci["bases"]
        nchunks = ci["nchunks"]
        n = ci["hi"] - ci["lo"]

        col_of_slot = ci["col_of_slot"].copy()
        unc = ci["uncovered"]
        if len(unc):
            cols = WS * nchunks + np.arange(len(unc))
            assert cols.max() < C
            col_of_slot[unc] = cols
        assert (col_of_slot >= 0).all()

        u_own = u[k * U_PER:(k + 1) * U_PER]     # [6250, 64]
        Au = (u_own @ g_w1[0:F_DIM]).astype(np.float32)   # [6250, 128]

        x_T = np.zeros((128, NE), np.float32)    # [vT ; eT ; onehot]
        lt = np.zeros((128, 128 * B), np.float32)
        oh4 = np.zeros((GP, WS * GROUPS * B), np.float32)
        deg = np.zeros(C, np.float32)
        if n:
            v_src = v[ci["sv"]].T                # [64, n]
            e_src = ci["se"].T                   # [16, n]
        for c in range(nchunks):
            s0, s1 = starts[c], starts[c + 1]
            m = s1 - s0
            base = bases[c]
            x_T[0:64, c * CHUNK:c * CHUNK + m] = v_src[:, s0:s1]
            x_T[64:80, c * CHUNK:c * CHUNK + m] = e_src[:, s0:s1]
            rel = su_l[s0:s1] - base
            j = np.arange(m)
            x_T[80 + rel, c * CHUNK + j] = 1.0
            lt[0:64, 128 * c:128 * (c + 1)] = W1v
            lt[64:80, 128 * c:128 * (c + 1)] = W1e
            hiu = min(base + WS, U_PER)
            lt[80:80 + hiu - base, 128 * c:128 * (c + 1)] = Au[base:hiu]
            oh4[j % GP, WS * GROUPS * c + WS * (j // GP) + rel] = 1.0
            deg[WS * c:WS * c + WS] = np.bincount(rel, minlength=WS)[:WS]

        u_T_compact = np.zeros((64, C), np.float32)
        u_T_compact[:, col_of_slot] = u_own.T

        per_core.append(dict(x_T=x_T, lt=lt, oh4=oh4,
                             u_T_compact=u_T_compact, deg=deg,
                             col_of_slot=col_of_slot))
    return per_core, B, NE, C


# ---------------------------------------------------------------- device side

def _build_program(B, NE, C, io_dtype_np, x_dtype_np, has_b2):
    import concourse.bacc as bacc
    import concourse.mybir as mybir
    import concourse.tile as tile

    FB = (C + 511) // 512               # f-MLP chunks
    MB = B // 2                         # blocks of 2 chunks
    md = mybir.dt.from_np(np.dtype(io_dtype_np))
    xd = mybir.dt.from_np(np.dtype(x_dtype_np))
    f32 = mybir.dt.float32
    Relu = mybir.ActivationFunctionType.Relu
    Alu = mybir.AluOpType

    nc = bacc.Bacc("TRN2", target_bir_lowering=False, debug=False,
                   num_devices=NCORES)

    # I/O
    x_T = nc.dram_tensor("x_T", [128, NE], xd, kind="ExternalInput")
    lt = nc.dram_tensor("lt", [128, 128 * B], md, kind="ExternalInput")
    oh4 = nc.dram_tensor("oh4", [GP, WS * GROUPS * B], xd,
                         kind="ExternalInput")
    if has_b2:
        corr = nc.dram_tensor("corr", [64, C], md, kind="ExternalInput")
    u_Tc = nc.dram_tensor("u_Tc", [64, C], md, kind="ExternalInput")
    w2 = nc.dram_tensor("w2", [128, 64], md, kind="ExternalInput")
    fw1 = nc.dram_tensor("fw1", [128, 128], md, kind="ExternalInput")
    fw2 = nc.dram_tensor("fw2", [128, 64], md, kind="ExternalInput")
    b1 = nc.dram_tensor("b1", [128, 1], f32, kind="ExternalInput")
    b2negm = nc.dram_tensor("b2negm", [GP, 64 * GROUPS], f32,
                            kind="ExternalInput")
    fb1 = nc.dram_tensor("fb1", [128, 1], f32, kind="ExternalInput")
    fb2 = nc.dram_tensor("fb2", [64, 1], f32, kind="ExternalInput")
    out_T = nc.dram_tensor("out_T", [64, C], md, kind="ExternalOutput")

    OHW = WS * GROUPS                   # one-hot cols per chunk

    with tile.TileContext(nc) as tc:
        with (
            tc.tile_pool(name="consts", bufs=1) as cp,
            tc.tile_pool(name="xf", bufs=1) as xfp,
            tc.tile_pool(name="xin", bufs=3) as xp,
            tc.tile_pool(name="ltin", bufs=3) as ltp,
            tc.tile_pool(name="ohin", bufs=3) as ohp,
            tc.tile_pool(name="wk3", bufs=3) as wp3,
            tc.tile_pool(name="wk2", bufs=2) as wp2,
            tc.tile_pool(name="pz1", bufs=2, space="PSUM") as pz1,
            tc.tile_pool(name="pz2", bufs=2, space="PSUM") as pz2,
            tc.tile_pool(name="ppT", bufs=1, space="PSUM") as ppT,
            tc.tile_pool(name="pf", bufs=1, space="PSUM") as pf,
        ):
            # resident constants (small ones first so batch 0 isn't delayed)
            w2_s = cp.tile([128, 64], md)
            fw1_s = cp.tile([128, 128], md)
            fw2_s = cp.tile([128, 64], md)
            b1_s = cp.tile([128, 1], f32)
            b2negm_s = cp.tile([GP, 64 * GROUPS], f32)
            fb1_s = cp.tile([128, 1], f32)
            fb2_s = cp.tile([64, 1], f32)
            for dst, src in [(w2_s, w2), (fw1_s, fw1), (fw2_s, fw2),
                             (b1_s, b1), (b2negm_s, b2negm), (fb1_s, fb1),
                             (fb2_s, fb2)]:
                nc.sync.dma_start(dst[:], src[:])

            # 4-block (8-chunk) input batches, prefetched one batch ahead
            NBAT = (MB + 3) // 4

            def load_batch(bi):
                if bi >= NBAT:
                    return None
                wl = min(8 * 128, 128 * B - bi * 8 * 128)
                lt_t = ltp.tile([128, 8 * 128], md, tag="lt")
                nc.sync.dma_start(lt_t[:, :wl],
                                  lt[:, bi * 8 * 128:bi * 8 * 128 + wl])
                w = min(8 * CHUNK, NE - bi * 8 * CHUNK)
                xt = xp.tile([128, 8 * CHUNK], xd, tag="x1")
                nc.sync.dma_start(xt[:, :w],
                                  x_T[:, bi * 8 * CHUNK:bi * 8 * CHUNK + w])
                wo = min(8 * OHW, OHW * B - bi * 8 * OHW)
                oh_t = ohp.tile([GP, 8 * OHW], xd, tag="oh4")
                nc.sync.dma_start(oh_t[:, :wo],
                                  oh4[:, bi * 8 * OHW:bi * 8 * OHW + wo])
                return xt, lt_t, oh_t

            bats = {0: load_batch(0), 1: load_batch(1)}

            # bulk constants after the first two batches
            xf = xfp.tile([128, C], md)
            nc.sync.dma_start(xf[64:128, :], u_Tc[:])
            if has_b2:
                corr_s = cp.tile([64, C], md)
                nc.sync.dma_start(corr_s[:], corr[:])

            def issue_l1(m):
                """L1 matmuls for block m into one [128, 1024] PSUM tile."""
                bat = bats[(2 * m) // 8]
                z1 = pz1.tile([128, 2 * CHUNK], f32, tag="z1")
                for q in range(2):
                    off = (2 * m + q) % 8
                    nc.tensor.matmul(
                        z1[:, q * CHUNK:(q + 1) * CHUNK],
                        lhsT=bat[1][:, off * 128:(off + 1) * 128],
                        rhs=bat[0][:, off * CHUNK:(off + 1) * CHUNK],
                        start=True, stop=True)
                return z1

            # f-MLP chunk emitter (interleaved into the main loop)
            f_done = [0]

            def emit_f(fc):
                w = min(512, C - 512 * fc)
                fsl = slice(512 * fc, 512 * fc + w)
                zf = pf.tile([128, 512], f32, tag="zf")
                nc.tensor.matmul(zf[:, :w], lhsT=fw1_s[:], rhs=xf[:, fsl],
                                 start=True, stop=True)
                hf = wp2.tile([128, 512], md, tag="hf")
                nc.scalar.activation(hf[:, :w], zf[:, :w], Relu,
                                     bias=fb1_s[:])
                nc.tensor.matmul(zf[0:64, :w], lhsT=fw2_s[:], rhs=hf[:, :w],
                                 start=True, stop=True)
                ot = wp2.tile([64, 512], md, tag="ot")
                nc.vector.tensor_scalar(ot[:, :w], zf[0:64, :w], fb2_s[:],
                                        0.0, op0=Alu.add, op1=Alu.max)
                nc.sync.dma_start(out_T[:, fsl], ot[:, :w])
                f_done[0] = fc + 1

            def issue_h1(z1):
                h1 = wp3.tile([128, 2 * CHUNK], md, tag="h1")
                nc.scalar.activation(h1[:], z1[:], Relu, bias=b1_s[:])
                return h1

            z1_cur = issue_l1(0)
            h1_cur = issue_h1(z1_cur)

            for m in range(MB):
                if m % 4 == 0 and m > 0:
                    bi = m // 4
                    bats.pop(bi - 1, None)
                    bats[bi + 1] = load_batch(bi + 1)
                if m + 1 < MB:
                    z1_next = issue_l1(m + 1)
                    h1_next = issue_h1(z1_next)
                else:
                    z1_next = h1_next = None

                # tails for block m
                bat = bats[(2 * m) // 8]
                h1 = h1_cur
                z2 = pz2.tile([128, 512], f32, tag="z2")
                for half in range(2):
                    for g in range(GROUPS):
                        j = half * GROUPS + g
                        nc.tensor.matmul(
                            z2[:, 64 * j:64 * (j + 1)],
                            lhsT=h1[:, GP * j:GP * (j + 1)],
                            rhs=w2_s[:], start=True, stop=True)
                pT = ppT.tile([64, 2 * WS], f32, tag="pT")
                for q in range(2):
                    h2 = wp3.tile([GP, 256], md, tag="h2")
                    nc.vector.tensor_tensor(h2[:], z2[:, 256 * q:256 * (q + 1)],
                                            b2negm_s[:], op=Alu.max)
                    ohoff = ((2 * m + q) % 8) * OHW
                    for g in range(GROUPS):
                        nc.tensor.matmul(
                            pT[:, WS * q:WS * (q + 1)],
                            lhsT=h2[:, 64 * g:64 * (g + 1)],
                            rhs=bat[2][:, ohoff + WS * g:ohoff + WS * (g + 1)],
                            start=(g == 0), stop=(g == GROUPS - 1))
                if has_b2:
                    nc.vector.tensor_tensor(
                        xf[0:64, 2 * WS * m:2 * WS * (m + 1)], pT[:],
                        corr_s[:, 2 * WS * m:2 * WS * (m + 1)], op=Alu.add)
                else:
                    nc.vector.tensor_copy(
                        xf[0:64, 2 * WS * m:2 * WS * (m + 1)], pT[:])
                z1_cur, h1_cur = z1_next, h1_next
                while (f_done[0] + 1) * 512 <= (m + 1) * 2 * WS:
                    emit_f(f_done[0])

            for fc in range(f_done[0], FB):
                emit_f(fc)

    nc.compile()
    return nc


def _make_consts(g_w2, g_b1, g_b2, f_w1, f_b1, f_w2, f_b2, io_dtype_np):
    dt = io_dtype_np
    g_b2 = np.asarray(g_b2, np.float32)
    # f-MLP input is [aggT ; uT] (agg rows first), so permute f_w1 rows
    f_w1 = np.asarray(f_w1, np.float32)
    f_w1p = np.concatenate([f_w1[64:128], f_w1[0:64]], axis=0)
    return dict(
        w2=np.asarray(g_w2, np.float32).astype(dt),
        fw1=np.ascontiguousarray(f_w1p).astype(dt),
        fw2=np.asarray(f_w2, np.float32).astype(dt),
        b1=np.asarray(g_b1, np.float32).reshape(128, 1),
        b2negm=np.ascontiguousarray(
            np.tile(-g_b2[None, :], (GP, GROUPS))).astype(np.float32),
        fb1=np.asarray(f_b1, np.float32).reshape(128, 1),
        fb2=np.asarray(f_b2, np.float32).reshape(64, 1),
    )


_last_run_info = {}


def kernel(u, v, e_vals, e_idx_v, e_idx_u, g_w1, g_b1, g_w2, g_b2,
           f_w1, f_b1, f_w2, f_b2, _trace=False):
    import ml_dtypes
    from concourse import bass_utils

    io_dtype_np = ml_dtypes.bfloat16
    x_dtype_np = ml_dtypes.float8_e3m4

    g_b2f = np.asarray(g_b2, np.float32)
    has_b2 = bool(np.any(g_b2f))

    per_core, B, NE, C = _preprocess(u, v, e_vals, e_idx_v, e_idx_u,
                                     g_w1, io_dtype_np)
    consts = _make_consts(g_w2, g_b1, g_b2, f_w1, f_b1, f_w2, f_b2,
                          io_dtype_np)
    nc = _build_program(B, NE, C, io_dtype_np, x_dtype_np, has_b2)

    in_maps = []
    for pc in per_core:
        m = dict(
            x_T=np.clip(pc["x_T"], -15.0, 15.0).astype(x_dtype_np),
            lt=pc["lt"].astype(io_dtype_np),
            oh4=pc["oh4"].astype(x_dtype_np),
            u_Tc=pc["u_T_compact"].astype(io_dtype_np),
            **consts,
        )
        if has_b2:
            m["corr"] = (g_b2f[:, None] * pc["deg"][None, :]) \
                .astype(io_dtype_np)
        in_maps.append(m)

    res = bass_utils.run_bass_kernel_spmd(
        nc, in_maps, core_ids=list(range(NCORES)), trace=_trace)
    _last_run_info.clear()
    _last_run_info.update(B=B, NE=NE, C=C, res=res)

    out = np.zeros((U, 64), np.float32)
    for k in range(NCORES):
        out_T = np.asarray(res.results[k]["out_T"]).astype(np.float32)
        cols = per_core[k]["col_of_slot"]
        out[k * U_PER:(k + 1) * U_PER] = out_T[:, cols].T
    return out


# revision 3
# speedup vs baseline: 1.0376x; 1.0152x over previous
"""Trainium2 Bass kernel for nn_HalfConv_876173328516 (GNN message passing).

Strategy (v2)
-------------
Host: sort edges by e_idx_u; core k owns u rows [k*6250, (k+1)*6250) so the 8
cores are fully independent. Edges are packed into 512-edge chunks whose u
values span < 48 slots. Per chunk the host emits:
  - a [128, 512] moving stream  x = [v_e(64) ; e_vals_e(16) ; onehot48_e(48)]
  - a [128, 128] stationary     lt = [W1v(64) ; W1e(16) ; Au_window(48)]
where Au = u @ g_w1[:64] (the per-u L1 contribution, precomputed on host), so
ONE K=128 N=512 matmul computes the full g-MLP layer-1 preactivation:
  z1[:, e] = W1v.T v_e + W1e.T e_e + Au[slot(e)].

Device (per core, per 512-edge chunk):
  L1   z1 = lt_c.T @ x_c                  (one N=512 matmul)
       h1 = relu(z1 + b1)                 (ACT -> SBUF bf16)
  L2   per 128-edge group: z2T = h1_g.T @ W2   (4x N=64 matmuls)
       h2T = max(z2T, -b2)                (DVE; relu(z+b) = max(z,-b)+b,
                                           the +deg*b2 lands in the flush)
  SUM  oh[e, slot] one-hot streamed from host (bf16)
       pT[64 feats, 48 slots] += h2T_g.T @ oh_g  (4x N=48, PSUM accumulate)
       xf[0:64, cols] = pT + b2*deg       (DVE add flush, bf16)
  f-MLP over compact slot columns: xf = [aggT ; uT], two matmuls + relus.

L1 for block m+1 is issued ahead of block m's tails so the PE never starves.

Host: out[u] = out_T[:, col_of_slot[u]].T per core.
"""

import numpy as np

U, V, E = 50000, 50000, 800000
NCORES = 8
U_PER = U // NCORES          # 6250
CHUNK = 512                  # edges per chunk
GP = 128                     # edges per matmul group
GROUPS = CHUNK // GP         # 4
WS = 48                      # slot window per chunk
F_DIM, G_DIM, H_DIM = 64, 64, 16


# ---------------------------------------------------------------- host side

def _preprocess(u, v, e_vals, e_idx_v, e_idx_u, g_w1, io_dtype_np):
    u = np.ascontiguousarray(np.asarray(u, np.float32))
    v = np.ascontiguousarray(np.asarray(v, np.float32))
    e_vals = np.ascontiguousarray(np.asarray(e_vals, np.float32))
    e_idx_u = np.asarray(e_idx_u).astype(np.int64)
    e_idx_v = np.asarray(e_idx_v).astype(np.int64)
    g_w1 = np.asarray(g_w1, np.float32)

    perm = np.argsort(e_idx_u, kind="stable")
    su = e_idx_u[perm]
    sv = e_idx_v[perm]
    se = e_vals[perm]

    core_lo = np.searchsorted(su, np.arange(NCORES) * U_PER, side="left")
    core_hi = np.searchsorted(su, (np.arange(NCORES) + 1) * U_PER, side="left")

    cores = []
    for k in range(NCORES):
        lo, hi = int(core_lo[k]), int(core_hi[k])
        su_l = (su[lo:hi] - k * U_PER).astype(np.int64)
        n = hi - lo
        starts, bases = [], []
        i = 0
        while i < n:
            base = int(su_l[i])
            j = min(i + CHUNK, n)
            j = min(j, int(np.searchsorted(su_l, base + WS, side="left")))
            if j < n:
                # step back to a u-boundary so no u straddles chunks
                j2 = int(np.searchsorted(su_l, su_l[j - 1], side="left"))
                if j2 > i and su_l[j - 1] == su_l[j]:
                    j = j2
            assert j > i, "u degree >= CHUNK unsupported"
            starts.append(i)
            bases.append(base)
            i = j
        starts.append(n)
        nchunks = len(bases)

        col_of_slot = np.full(U_PER, -1, np.int64)
        for c in range(nchunks):
            s0, s1 = starts[c], starts[c + 1]
            slots = np.unique(su_l[s0:s1])
            assert slots.max() - bases[c] < WS
            col_of_slot[slots] = WS * c + (slots - bases[c])
        uncovered = np.flatnonzero(col_of_slot < 0)
        cores.append(dict(lo=lo, hi=hi, su_l=su_l, sv=sv[lo:hi],
                          se=se[lo:hi], starts=starts, bases=bases,
                          nchunks=nchunks, col_of_slot=col_of_slot,
                          uncovered=uncovered))

    need = max(c["nchunks"] + (len(c["uncovered"]) + WS - 1) // WS + 1
               for c in cores)
    B = need + (need % 2)            # chunks, in blocks of 2
    NE = B * CHUNK
    C = B * WS

    W1v = g_w1[F_DIM:F_DIM + G_DIM]              # [64, 128]
    W1e = g_w1[F_DIM + G_DIM:]                   # [16, 128]

    per_core = []
    for k in range(NCORES):
        ci = cores[k]
        su_l, starts, bases = ci["su_l"], ci["starts"], ci["bases"]
        nchunks = ci["nchunks"]
        n = ci["hi"] - ci["lo"]

        col_of_slot = ci["col_of_slot"].copy()
        unc = ci["uncovered"]
        if len(unc):
            cols = WS * nchunks + np.arange(len(unc))
            assert cols.max() < C
            col_of_slot[unc] = cols
        assert (col_of_slot >= 0).all()

        u_own = u[k * U_PER:(k + 1) * U_PER]     # [6250, 64]
        Au = (u_own @ g_w1[0:F_DIM]).astype(np.float32)   # [6250, 128]

        x_T = np.zeros((128, NE), np.float32)    # [vT ; eT ; onehot]
        lt = np.zeros((128, 128 * B), np.float32)
        oh4 = np.zeros((GP, WS * GROUPS * B), np.float32)
        deg = np.zeros(C, np.float32)
        if n:
            v_src = v[ci["sv"]].T                # [64, n]
            e_src = ci["se"].T                   # [16, n]
        for c in range(nchunks):
            s0, s1 = starts[c], starts[c + 1]
            m = s1 - s0
            base = bases[c]
            x_T[0:64, c * CHUNK:c * CHUNK + m] = v_src[:, s0:s1]
            x_T[64:80, c * CHUNK:c * CHUNK + m] = e_src[:, s0:s1]
            rel = su_l[s0:s1] - base
            j = np.arange(m)
            x_T[80 + rel, c * CHUNK + j] = 1.0
            lt[0:64, 128 * c:128 * (c + 1)] = W1v
            lt[64:80, 128 * c:128 * (c + 1)] = W1e
            hiu = min(base + WS, U_PER)
            lt[80:80 + hiu - base, 128 * c:128 * (c + 1)] = Au[base:hiu]
            oh4[j % GP, WS * GROUPS * c + WS * (j // GP) + rel] = 1.0
            deg[WS * c:WS * c + WS] = np.bincount(rel, minlength=WS)[:WS]

        u_T_compact = np.zeros((64, C), np.float32)
        u_T_compact[:, col_of_slot] = u_own.T

        per_core.append(dict(x_T=x_T, lt=lt, oh4=oh4,
                             u_T_compact=u_T_compact, deg=deg,
                             col_of_slot=col_of_slot))
    return per_core, B, NE, C


# ---------------------------------------------------------------- device side

def _build_program(B, NE, C, io_dtype_np, x_dtype_np, has_b2):
    import concourse.bacc as bacc
    import concourse.mybir as mybir
    import concourse.tile as tile

    FB = (C + 511) // 512               # f-MLP chunks
    MB = B // 2                         # blocks of 2 chunks
    md = mybir.dt.from_np(np.dtype(io_dtype_np))
    xd = mybir.dt.from_np(np.dtype(x_dtype_np))
    f32 = mybir.dt.float32
    Relu = mybir.ActivationFunctionType.Relu
    Alu = mybir.AluOpType

    nc = bacc.Bacc("TRN2", target_bir_lowering=False, debug=False,
                   num_devices=NCORES)

    # I/O
    x_T = nc.dram_tensor("x_T", [128, NE], xd, kind="ExternalInput")
    lt = nc.dram_tensor("lt", [128, 128 * B], md, kind="ExternalInput")
    oh4 = nc.dram_tensor("oh4", [GP, WS * GROUPS * B], xd,
                         kind="ExternalInput")
    if has_b2:
        corr = nc.dram_tensor("corr", [64, C], md, kind="ExternalInput")
    u_Tc = nc.dram_tensor("u_Tc", [64, C], md, kind="ExternalInput")
    w2 = nc.dram_tensor("w2", [128, 64], md, kind="ExternalInput")
    fw1 = nc.dram_tensor("fw1", [128, 128], md, kind="ExternalInput")
    fw2 = nc.dram_tensor("fw2", [128, 64], md, kind="ExternalInput")
    b1 = nc.dram_tensor("b1", [128, 1], f32, kind="ExternalInput")
    b2negm = nc.dram_tensor("b2negm", [GP, 64 * GROUPS], f32,
                            kind="ExternalInput")
    fb1 = nc.dram_tensor("fb1", [128, 1], f32, kind="ExternalInput")
    fb2 = nc.dram_tensor("fb2", [64, 1], f32, kind="ExternalInput")
    out_T = nc.dram_tensor("out_T", [64, C], md, kind="ExternalOutput")

    OHW = WS * GROUPS                   # one-hot cols per chunk

    with tile.TileContext(nc) as tc:
        with (
            tc.tile_pool(name="consts", bufs=1) as cp,
            tc.tile_pool(name="xf", bufs=1) as xfp,
            tc.tile_pool(name="xin", bufs=3) as xp,
            tc.tile_pool(name="ltin", bufs=3) as ltp,
            tc.tile_pool(name="ohin", bufs=3) as ohp,
            tc.tile_pool(name="wk3", bufs=3) as wp3,
            tc.tile_pool(name="wk2", bufs=2) as wp2,
            tc.tile_pool(name="pz1", bufs=2, space="PSUM") as pz1,
            tc.tile_pool(name="pz2", bufs=2, space="PSUM") as pz2,
            tc.tile_pool(name="ppT", bufs=1, space="PSUM") as ppT,
            tc.tile_pool(name="pf", bufs=1, space="PSUM") as pf,
        ):
            # input batches of 16 chunks (8 blocks), issued from the idle
            # GpSimd queue so descriptor writes overlap the const loads that
            # the Sync queue issues concurrently
            BCH = 8                     # chunks per batch
            NBAT = (B + BCH - 1) // BCH

            def load_batch(bi):
                if bi >= NBAT:
                    return None
                wl = min(BCH * 128, 128 * B - bi * BCH * 128)
                lt_t = ltp.tile([128, BCH * 128], md, tag="lt")
                nc.gpsimd.dma_start(
                    lt_t[:, :wl], lt[:, bi * BCH * 128:bi * BCH * 128 + wl])
                w = min(BCH * CHUNK, NE - bi * BCH * CHUNK)
                xt = xp.tile([128, BCH * CHUNK], xd, tag="x1")
                nc.gpsimd.dma_start(
                    xt[:, :w], x_T[:, bi * BCH * CHUNK:bi * BCH * CHUNK + w])
                wo = min(BCH * OHW, OHW * B - bi * BCH * OHW)
                oh_t = ohp.tile([GP, BCH * OHW], xd, tag="oh4")
                nc.gpsimd.dma_start(
                    oh_t[:, :wo], oh4[:, bi * BCH * OHW:bi * BCH * OHW + wo])
                return xt, lt_t, oh_t

            bat0 = load_batch(0)

            # resident constants (on the Sync queue, parallel with batch 0)
            w2_s = cp.tile([128, 64], md)
            b1_s = cp.tile([128, 1], f32)
            b2negm_s = cp.tile([GP, 64 * GROUPS], f32)
            fw1_s = cp.tile([128, 128], md)
            fw2_s = cp.tile([128, 64], md)
            fb1_s = cp.tile([128, 1], f32)
            fb2_s = cp.tile([64, 1], f32)
            for dst, src in [(w2_s, w2), (b1_s, b1), (b2negm_s, b2negm),
                             (fw1_s, fw1), (fw2_s, fw2), (fb1_s, fb1),
                             (fb2_s, fb2)]:
                nc.sync.dma_start(dst[:], src[:])

            bats = {0: bat0, 1: load_batch(1)}

            # bulk constants after the first two batches
            xf = xfp.tile([128, C], md)
            nc.sync.dma_start(xf[64:128, :], u_Tc[:])
            if has_b2:
                corr_s = cp.tile([64, C], md)
                nc.sync.dma_start(corr_s[:], corr[:])

            def issue_l1(m):
                """L1 matmuls for block m into one [128, 1024] PSUM tile."""
                bat = bats[(2 * m) // BCH]
                z1 = pz1.tile([128, 2 * CHUNK], f32, tag="z1")
                for q in range(2):
                    off = (2 * m + q) % BCH
                    nc.tensor.matmul(
                        z1[:, q * CHUNK:(q + 1) * CHUNK],
                        lhsT=bat[1][:, off * 128:(off + 1) * 128],
                        rhs=bat[0][:, off * CHUNK:(off + 1) * CHUNK],
                        start=True, stop=True)
                return z1

            # f-MLP chunk emitter (interleaved into the main loop)
            f_done = [0]

            def emit_f(fc):
                w = min(512, C - 512 * fc)
                fsl = slice(512 * fc, 512 * fc + w)
                zf = pf.tile([128, 512], f32, tag="zf")
                nc.tensor.matmul(zf[:, :w], lhsT=fw1_s[:], rhs=xf[:, fsl],
                                 start=True, stop=True)
                hf = wp2.tile([128, 512], md, tag="hf")
                nc.scalar.activation(hf[:, :w], zf[:, :w], Relu,
                                     bias=fb1_s[:])
                nc.tensor.matmul(zf[0:64, :w], lhsT=fw2_s[:], rhs=hf[:, :w],
                                 start=True, stop=True)
                ot = wp2.tile([64, 512], md, tag="ot")
                nc.vector.tensor_scalar(ot[:, :w], zf[0:64, :w], fb2_s[:],
                                        0.0, op0=Alu.add, op1=Alu.max)
                nc.sync.dma_start(out_T[:, fsl], ot[:, :w])
                f_done[0] = fc + 1

            def issue_h1(z1):
                h1 = wp3.tile([128, 2 * CHUNK], md, tag="h1")
                nc.scalar.activation(h1[:], z1[:], Relu, bias=b1_s[:])
                return h1

            z1_cur = issue_l1(0)
            h1_cur = issue_h1(z1_cur)

            for m in range(MB):
                if m % (BCH // 2) == 0 and m > 0:
                    bi = m // (BCH // 2)
                    bats.pop(bi - 1, None)
                    bats[bi + 1] = load_batch(bi + 1)
                if m + 1 < MB:
                    z1_next = issue_l1(m + 1)
                    h1_next = issue_h1(z1_next)
                else:
                    z1_next = h1_next = None

                # tails for block m
                bat = bats[(2 * m) // BCH]
                h1 = h1_cur
                z2 = pz2.tile([128, 512], f32, tag="z2")
                for half in range(2):
                    for g in range(GROUPS):
                        j = half * GROUPS + g
                        nc.tensor.matmul(
                            z2[:, 64 * j:64 * (j + 1)],
                            lhsT=h1[:, GP * j:GP * (j + 1)],
                            rhs=w2_s[:], start=True, stop=True)
                pT = ppT.tile([64, 2 * WS], f32, tag="pT")
                for q in range(2):
                    h2 = wp3.tile([GP, 256], md, tag="h2")
                    nc.vector.tensor_tensor(h2[:], z2[:, 256 * q:256 * (q + 1)],
                                            b2negm_s[:], op=Alu.max)
                    ohoff = ((2 * m + q) % BCH) * OHW
                    for g in range(GROUPS):
                        nc.tensor.matmul(
                            pT[:, WS * q:WS * (q + 1)],
                            lhsT=h2[:, 64 * g:64 * (g + 1)],
                            rhs=bat[2][:, ohoff + WS * g:ohoff + WS * (g + 1)],
                            start=(g == 0), stop=(g == GROUPS - 1))
                if has_b2:
                    nc.vector.tensor_tensor(
                        xf[0:64, 2 * WS * m:2 * WS * (m + 1)], pT[:],
                        corr_s[:, 2 * WS * m:2 * WS * (m + 1)], op=Alu.add)
                else:
                    nc.vector.tensor_copy(
                        xf[0:64, 2 * WS * m:2 * WS * (m + 1)], pT[:])
                z1_cur, h1_cur = z1_next, h1_next
                while (f_done[0] + 1) * 512 <= (m + 1) * 2 * WS:
                    emit_f(f_done[0])

            for fc in range(f_done[0], FB):
                emit_f(fc)

    nc.compile()
    return nc


def _make_consts(g_w2, g_b1, g_b2, f_w1, f_b1, f_w2, f_b2, io_dtype_np):
    dt = io_dtype_np
    g_b2 = np.asarray(g_b2, np.float32)
    # f-MLP input is [aggT ; uT] (agg rows first), so permute f_w1 rows
    f_w1 = np.asarray(f_w1, np.float32)
    f_w1p = np.concatenate([f_w1[64:128], f_w1[0:64]], axis=0)
    return dict(
        w2=np.asarray(g_w2, np.float32).astype(dt),
        fw1=np.ascontiguousarray(f_w1p).astype(dt),
        fw2=np.asarray(f_w2, np.float32).astype(dt),
        b1=np.asarray(g_b1, np.float32).reshape(128, 1),
        b2negm=np.ascontiguousarray(
            np.tile(-g_b2[None, :], (GP, GROUPS))).astype(np.float32),
        fb1=np.asarray(f_b1, np.float32).reshape(128, 1),
        fb2=np.asarray(f_b2, np.float32).reshape(64, 1),
    )


_last_run_info = {}


def kernel(u, v, e_vals, e_idx_v, e_idx_u, g_w1, g_b1, g_w2, g_b2,
           f_w1, f_b1, f_w2, f_b2, _trace=False):
    import ml_dtypes
    from concourse import bass_utils

    io_dtype_np = ml_dtypes.bfloat16
    x_dtype_np = ml_dtypes.float8_e3m4

    g_b2f = np.asarray(g_b2, np.float32)
    has_b2 = bool(np.any(g_b2f))

    per_core, B, NE, C = _preprocess(u, v, e_vals, e_idx_v, e_idx_u,
                                     g_w1, io_dtype_np)
    consts = _make_consts(g_w2, g_b1, g_b2, f_w1, f_b1, f_w2, f_b2,
                          io_dtype_np)
    nc = _build_program(B, NE, C, io_dtype_np, x_dtype_np, has_b2)

    in_maps = []
    for pc in per_core:
        m = dict(
            x_T=np.clip(pc["x_T"], -15.0, 15.0).astype(x_dtype_np),
            lt=pc["lt"].astype(io_dtype_np),
            oh4=pc["oh4"].astype(x_dtype_np),
            u_Tc=pc["u_T_compact"].astype(io_dtype_np),
            **consts,
        )
        if has_b2:
            m["corr"] = (g_b2f[:, None] * pc["deg"][None, :]) \
                .astype(io_dtype_np)
        in_maps.append(m)

    res = bass_utils.run_bass_kernel_spmd(
        nc, in_maps, core_ids=list(range(NCORES)), trace=_trace)
    _last_run_info.clear()
    _last_run_info.update(B=B, NE=NE, C=C, res=res)

    out = np.zeros((U, 64), np.float32)
    for k in range(NCORES):
        out_T = np.asarray(res.results[k]["out_T"]).astype(np.float32)
        cols = per_core[k]["col_of_slot"]
        out[k * U_PER:(k + 1) * U_PER] = out_T[:, cols].T
    return out


# revision 4
# speedup vs baseline: 1.1627x; 1.1205x over previous
"""Trainium2 Bass kernel for nn_HalfConv_876173328516 (GNN message passing).

Strategy (v2)
-------------
Host: sort edges by e_idx_u; core k owns u rows [k*6250, (k+1)*6250) so the 8
cores are fully independent. Edges are packed into 512-edge chunks whose u
values span < 48 slots. Per chunk the host emits:
  - a [128, 512] moving stream  x = [v_e(64) ; e_vals_e(16) ; onehot48_e(48)]
  - a [128, 128] stationary     lt = [W1v(64) ; W1e(16) ; Au_window(48)]
where Au = u @ g_w1[:64] (the per-u L1 contribution, precomputed on host), so
ONE K=128 N=512 matmul computes the full g-MLP layer-1 preactivation:
  z1[:, e] = W1v.T v_e + W1e.T e_e + Au[slot(e)].

Device (per core, per 512-edge chunk):
  L1   z1 = lt_c.T @ x_c                  (one N=512 matmul)
       h1 = relu(z1 + b1)                 (ACT -> SBUF bf16)
  L2   per 128-edge group: z2T = h1_g.T @ W2   (4x N=64 matmuls)
       h2T = max(z2T, -b2)                (DVE; relu(z+b) = max(z,-b)+b,
                                           the +deg*b2 lands in the flush)
  SUM  oh[e, slot] one-hot streamed from host (bf16)
       pT[64 feats, 48 slots] += h2T_g.T @ oh_g  (4x N=48, PSUM accumulate)
       xf[0:64, cols] = pT + b2*deg       (DVE add flush, bf16)
  f-MLP over compact slot columns: xf = [aggT ; uT], two matmuls + relus.

L1 for block m+1 is issued ahead of block m's tails so the PE never starves.

Host: out[u] = out_T[:, col_of_slot[u]].T per core.
"""

import numpy as np

U, V, E = 50000, 50000, 800000
NCORES = 8
U_PER = U // NCORES          # 6250
CHUNK = 512                  # edges per chunk
GP = 128                     # edges per matmul group
GROUPS = CHUNK // GP         # 4
WS = 48                      # slot window per chunk
F_DIM, G_DIM, H_DIM = 64, 64, 16


# ---------------------------------------------------------------- host side

def _preprocess(u, v, e_vals, e_idx_v, e_idx_u, g_w1, io_dtype_np):
    u = np.ascontiguousarray(np.asarray(u, np.float32))
    v = np.ascontiguousarray(np.asarray(v, np.float32))
    e_vals = np.ascontiguousarray(np.asarray(e_vals, np.float32))
    e_idx_u = np.asarray(e_idx_u).astype(np.int64)
    e_idx_v = np.asarray(e_idx_v).astype(np.int64)
    g_w1 = np.asarray(g_w1, np.float32)

    perm = np.argsort(e_idx_u, kind="stable")
    su = e_idx_u[perm]
    sv = e_idx_v[perm]
    se = e_vals[perm]

    core_lo = np.searchsorted(su, np.arange(NCORES) * U_PER, side="left")
    core_hi = np.searchsorted(su, (np.arange(NCORES) + 1) * U_PER, side="left")

    cores = []
    for k in range(NCORES):
        lo, hi = int(core_lo[k]), int(core_hi[k])
        su_l = (su[lo:hi] - k * U_PER).astype(np.int64)
        n = hi - lo
        starts, bases = [], []
        i = 0
        while i < n:
            base = int(su_l[i])
            j = min(i + CHUNK, n)
            j = min(j, int(np.searchsorted(su_l, base + WS, side="left")))
            if j < n:
                # step back to a u-boundary so no u straddles chunks
                j2 = int(np.searchsorted(su_l, su_l[j - 1], side="left"))
                if j2 > i and su_l[j - 1] == su_l[j]:
                    j = j2
            assert j > i, "u degree >= CHUNK unsupported"
            starts.append(i)
            bases.append(base)
            i = j
        starts.append(n)
        nchunks = len(bases)

        col_of_slot = np.full(U_PER, -1, np.int64)
        for c in range(nchunks):
            s0, s1 = starts[c], starts[c + 1]
            slots = np.unique(su_l[s0:s1])
            assert slots.max() - bases[c] < WS
            col_of_slot[slots] = WS * c + (slots - bases[c])
        uncovered = np.flatnonzero(col_of_slot < 0)
        cores.append(dict(lo=lo, hi=hi, su_l=su_l, sv=sv[lo:hi],
                          se=se[lo:hi], starts=starts, bases=bases,
                          nchunks=nchunks, col_of_slot=col_of_slot,
                          uncovered=uncovered))

    need = max(c["nchunks"] + (len(c["uncovered"]) + WS - 1) // WS
               for c in cores)
    B = need + (need % 2)            # chunks, in blocks of 2
    NE = B * CHUNK
    C = B * WS

    W1v = g_w1[F_DIM:F_DIM + G_DIM]              # [64, 128]
    W1e = g_w1[F_DIM + G_DIM:]                   # [16, 128]

    per_core = []
    for k in range(NCORES):
        ci = cores[k]
        su_l, starts, bases = ci["su_l"], ci["starts"], ci["bases"]
        nchunks = ci["nchunks"]
        n = ci["hi"] - ci["lo"]

        col_of_slot = ci["col_of_slot"].copy()
        unc = ci["uncovered"]
        if len(unc):
            cols = WS * nchunks + np.arange(len(unc))
            assert cols.max() < C
            col_of_slot[unc] = cols
        assert (col_of_slot >= 0).all()

        u_own = u[k * U_PER:(k + 1) * U_PER]     # [6250, 64]
        Au = (u_own @ g_w1[0:F_DIM]).astype(np.float32)   # [6250, 128]

        x_T = np.zeros((128, NE), np.float32)    # [vT ; eT ; onehot]
        lt = np.zeros((128, 128 * B), np.float32)
        oh4 = np.zeros((GP, WS * GROUPS * B), np.float32)
        deg = np.zeros(C, np.float32)
        if n:
            v_src = v[ci["sv"]].T                # [64, n]
            e_src = ci["se"].T                   # [16, n]
        for c in range(nchunks):
            s0, s1 = starts[c], starts[c + 1]
            m = s1 - s0
            base = bases[c]
            x_T[0:64, c * CHUNK:c * CHUNK + m] = v_src[:, s0:s1]
            x_T[64:80, c * CHUNK:c * CHUNK + m] = e_src[:, s0:s1]
            rel = su_l[s0:s1] - base
            j = np.arange(m)
            x_T[80 + rel, c * CHUNK + j] = 1.0
            lt[0:64, 128 * c:128 * (c + 1)] = W1v
            lt[64:80, 128 * c:128 * (c + 1)] = W1e
            hiu = min(base + WS, U_PER)
            lt[80:80 + hiu - base, 128 * c:128 * (c + 1)] = Au[base:hiu]
            oh4[j % GP, WS * GROUPS * c + WS * (j // GP) + rel] = 1.0
            deg[WS * c:WS * c + WS] = np.bincount(rel, minlength=WS)[:WS]

        u_T_compact = np.zeros((64, C), np.float32)
        u_T_compact[:, col_of_slot] = u_own.T

        per_core.append(dict(x_T=x_T, lt=lt, oh4=oh4,
                             u_T_compact=u_T_compact, deg=deg,
                             col_of_slot=col_of_slot))
    return per_core, B, NE, C


# ---------------------------------------------------------------- device side

def _build_program(B, NE, C, io_dtype_np, x_dtype_np, has_b2):
    import concourse.bacc as bacc
    import concourse.mybir as mybir
    import concourse.tile as tile

    FB = (C + 511) // 512               # f-MLP chunks
    MB = B // 2                         # blocks of 2 chunks
    md = mybir.dt.from_np(np.dtype(io_dtype_np))
    xd = mybir.dt.from_np(np.dtype(x_dtype_np))
    f32 = mybir.dt.float32
    Relu = mybir.ActivationFunctionType.Relu
    Alu = mybir.AluOpType

    nc = bacc.Bacc("TRN2", target_bir_lowering=False, debug=False,
                   num_devices=NCORES)

    # I/O
    x_T = nc.dram_tensor("x_T", [128, NE], xd, kind="ExternalInput")
    lt = nc.dram_tensor("lt", [128, 128 * B], md, kind="ExternalInput")
    oh4 = nc.dram_tensor("oh4", [GP, WS * GROUPS * B], xd,
                         kind="ExternalInput")
    if has_b2:
        corr = nc.dram_tensor("corr", [64, C], md, kind="ExternalInput")
    u_Tc = nc.dram_tensor("u_Tc", [64, C], md, kind="ExternalInput")
    w2 = nc.dram_tensor("w2", [128, 64], md, kind="ExternalInput")
    fw1 = nc.dram_tensor("fw1", [128, 128], md, kind="ExternalInput")
    fw2 = nc.dram_tensor("fw2", [128, 64], md, kind="ExternalInput")
    b1 = nc.dram_tensor("b1", [128, 1], f32, kind="ExternalInput")
    b2negm = nc.dram_tensor("b2negm", [GP, 64 * GROUPS], f32,
                            kind="ExternalInput")
    fb1 = nc.dram_tensor("fb1", [128, 1], f32, kind="ExternalInput")
    fb2 = nc.dram_tensor("fb2", [64, 1], f32, kind="ExternalInput")
    out_T = nc.dram_tensor("out_T", [64, C], md, kind="ExternalOutput")

    OHW = WS * GROUPS                   # one-hot cols per chunk

    with tile.TileContext(nc) as tc:
        with (
            tc.tile_pool(name="consts", bufs=1) as cp,
            tc.tile_pool(name="xf", bufs=1) as xfp,
            tc.tile_pool(name="xin", bufs=3) as xp,
            tc.tile_pool(name="ltin", bufs=3) as ltp,
            tc.tile_pool(name="ohin", bufs=3) as ohp,
            tc.tile_pool(name="wk3", bufs=3) as wp3,
            tc.tile_pool(name="wk2", bufs=2) as wp2,
            tc.tile_pool(name="pz1", bufs=2, space="PSUM") as pz1,
            tc.tile_pool(name="pz2", bufs=2, space="PSUM") as pz2,
            tc.tile_pool(name="ppT", bufs=1, space="PSUM") as ppT,
            tc.tile_pool(name="pf", bufs=1, space="PSUM") as pf,
        ):
            # input batches of 16 chunks (8 blocks), issued from the idle
            # GpSimd queue so descriptor writes overlap the const loads that
            # the Sync queue issues concurrently
            BCH = 8                     # chunks per batch
            NBAT = (B + BCH - 1) // BCH

            def load_batch(bi):
                if bi >= NBAT:
                    return None
                wl = min(BCH * 128, 128 * B - bi * BCH * 128)
                lt_t = ltp.tile([128, BCH * 128], md, tag="lt")
                nc.gpsimd.dma_start(
                    lt_t[:, :wl], lt[:, bi * BCH * 128:bi * BCH * 128 + wl])
                w = min(BCH * CHUNK, NE - bi * BCH * CHUNK)
                xt = xp.tile([128, BCH * CHUNK], xd, tag="x1")
                nc.gpsimd.dma_start(
                    xt[:, :w], x_T[:, bi * BCH * CHUNK:bi * BCH * CHUNK + w])
                wo = min(BCH * OHW, OHW * B - bi * BCH * OHW)
                oh_t = ohp.tile([GP, BCH * OHW], xd, tag="oh4")
                nc.gpsimd.dma_start(
                    oh_t[:, :wo], oh4[:, bi * BCH * OHW:bi * BCH * OHW + wo])
                return xt, lt_t, oh_t

            bat0 = load_batch(0)

            # resident constants (on the Sync queue, parallel with batch 0)
            w2_s = cp.tile([128, 64], md)
            b1_s = cp.tile([128, 1], f32)
            b2negm_s = cp.tile([GP, 64 * GROUPS], f32)
            fw1_s = cp.tile([128, 128], md)
            fw2_s = cp.tile([128, 64], md)
            fb1_s = cp.tile([128, 1], f32)
            fb2_s = cp.tile([64, 1], f32)
            for dst, src in [(w2_s, w2), (b1_s, b1), (b2negm_s, b2negm),
                             (fw1_s, fw1), (fw2_s, fw2), (fb1_s, fb1),
                             (fb2_s, fb2)]:
                nc.sync.dma_start(dst[:], src[:])

            bats = {0: bat0, 1: load_batch(1), 2: load_batch(2)}

            # bulk constants after the first two batches
            xf = xfp.tile([128, C], md)
            nc.sync.dma_start(xf[64:128, :], u_Tc[:])
            if has_b2:
                corr_s = cp.tile([64, C], md)
                nc.sync.dma_start(corr_s[:], corr[:])

            def issue_l1(m):
                """L1 matmuls for block m into one [128, 1024] PSUM tile."""
                bat = bats[(2 * m) // BCH]
                z1 = pz1.tile([128, 2 * CHUNK], f32, tag="z1")
                for q in range(2):
                    off = (2 * m + q) % BCH
                    nc.tensor.matmul(
                        z1[:, q * CHUNK:(q + 1) * CHUNK],
                        lhsT=bat[1][:, off * 128:(off + 1) * 128],
                        rhs=bat[0][:, off * CHUNK:(off + 1) * CHUNK],
                        start=True, stop=True)
                return z1

            # f-MLP chunk emitter (interleaved into the main loop)
            f_done = [0]

            def emit_f(fc):
                w = min(512, C - 512 * fc)
                fsl = slice(512 * fc, 512 * fc + w)
                zf = pf.tile([128, 512], f32, tag="zf")
                nc.tensor.matmul(zf[:, :w], lhsT=fw1_s[:], rhs=xf[:, fsl],
                                 start=True, stop=True)
                hf = wp2.tile([128, 512], md, tag="hf")
                nc.scalar.activation(hf[:, :w], zf[:, :w], Relu,
                                     bias=fb1_s[:])
                nc.tensor.matmul(zf[0:64, :w], lhsT=fw2_s[:], rhs=hf[:, :w],
                                 start=True, stop=True)
                ot = wp2.tile([64, 512], md, tag="ot")
                nc.vector.tensor_scalar(ot[:, :w], zf[0:64, :w], fb2_s[:],
                                        0.0, op0=Alu.add, op1=Alu.max)
                nc.sync.dma_start(out_T[:, fsl], ot[:, :w])
                f_done[0] = fc + 1

            def issue_h1(z1):
                h1 = wp3.tile([128, 2 * CHUNK], md, tag="h1")
                nc.scalar.activation(h1[:], z1[:], Relu, bias=b1_s[:])
                return h1

            z1_cur = issue_l1(0)
            h1_cur = issue_h1(z1_cur)

            for m in range(MB):
                if m % (BCH // 2) == 0 and m > 0:
                    bi = m // (BCH // 2)
                    bats.pop(bi - 1, None)
                    bats[bi + 2] = load_batch(bi + 2)
                if m + 1 < MB:
                    z1_next = issue_l1(m + 1)
                    h1_next = issue_h1(z1_next)
                else:
                    z1_next = h1_next = None

                # tails for block m
                bat = bats[(2 * m) // BCH]
                h1 = h1_cur
                z2 = pz2.tile([128, 512], f32, tag="z2")
                for half in range(2):
                    for g in range(GROUPS):
                        j = half * GROUPS + g
                        nc.tensor.matmul(
                            z2[:, 64 * j:64 * (j + 1)],
                            lhsT=h1[:, GP * j:GP * (j + 1)],
                            rhs=w2_s[:], start=True, stop=True)
                pT = ppT.tile([64, 2 * WS], f32, tag="pT")
                for q in range(2):
                    h2 = wp3.tile([GP, 256], md, tag="h2")
                    nc.vector.tensor_tensor(h2[:], z2[:, 256 * q:256 * (q + 1)],
                                            b2negm_s[:], op=Alu.max)
                    ohoff = ((2 * m + q) % BCH) * OHW
                    for g in range(GROUPS):
                        nc.tensor.matmul(
                            pT[:, WS * q:WS * (q + 1)],
                            lhsT=h2[:, 64 * g:64 * (g + 1)],
                            rhs=bat[2][:, ohoff + WS * g:ohoff + WS * (g + 1)],
                            start=(g == 0), stop=(g == GROUPS - 1))
                if has_b2:
                    nc.vector.tensor_tensor(
                        xf[0:64, 2 * WS * m:2 * WS * (m + 1)], pT[:],
                        corr_s[:, 2 * WS * m:2 * WS * (m + 1)], op=Alu.add)
                else:
                    nc.vector.tensor_copy(
                        xf[0:64, 2 * WS * m:2 * WS * (m + 1)], pT[:])
                z1_cur, h1_cur = z1_next, h1_next
                while (f_done[0] + 1) * 512 <= (m + 1) * 2 * WS:
                    emit_f(f_done[0])

            for fc in range(f_done[0], FB):
                emit_f(fc)

    nc.compile()
    return nc


def _make_consts(g_w2, g_b1, g_b2, f_w1, f_b1, f_w2, f_b2, io_dtype_np):
    dt = io_dtype_np
    g_b2 = np.asarray(g_b2, np.float32)
    # f-MLP input is [aggT ; uT] (agg rows first), so permute f_w1 rows
    f_w1 = np.asarray(f_w1, np.float32)
    f_w1p = np.concatenate([f_w1[64:128], f_w1[0:64]], axis=0)
    return dict(
        w2=np.asarray(g_w2, np.float32).astype(dt),
        fw1=np.ascontiguousarray(f_w1p).astype(dt),
        fw2=np.asarray(f_w2, np.float32).astype(dt),
        b1=np.asarray(g_b1, np.float32).reshape(128, 1),
        b2negm=np.ascontiguousarray(
            np.tile(-g_b2[None, :], (GP, GROUPS))).astype(np.float32),
        fb1=np.asarray(f_b1, np.float32).reshape(128, 1),
        fb2=np.asarray(f_b2, np.float32).reshape(64, 1),
    )


_last_run_info = {}


def kernel(u, v, e_vals, e_idx_v, e_idx_u, g_w1, g_b1, g_w2, g_b2,
           f_w1, f_b1, f_w2, f_b2, _trace=False):
    import ml_dtypes
    from concourse import bass_utils

    io_dtype_np = ml_dtypes.bfloat16
    x_dtype_np = ml_dtypes.float8_e3m4

    g_b2f = np.asarray(g_b2, np.float32)
    has_b2 = bool(np.any(g_b2f))

    per_core, B, NE, C = _preprocess(u, v, e_vals, e_idx_v, e_idx_u,
                                     g_w1, io_dtype_np)
    consts = _make_consts(g_w2, g_b1, g_b2, f_w1, f_b1, f_w2, f_b2,
                          io_dtype_np)
    nc = _build_program(B, NE, C, io_dtype_np, x_dtype_np, has_b2)

    in_maps = []
    for pc in per_core:
        m = dict(
            x_T=np.clip(pc["x_T"], -15.0, 15.0).astype(x_dtype_np),
            lt=pc["lt"].astype(io_dtype_np),
            oh4=pc["oh4"].astype(x_dtype_np),
            u_Tc=pc["u_T_compact"].astype(io_dtype_np),
            **consts,
        )
        if has_b2:
            m["corr"] = (g_b2f[:, None] * pc["deg"][None, :]) \
                .astype(io_dtype_np)
        in_maps.append(m)

    res = bass_utils.run_bass_kernel_spmd(
        nc, in_maps, core_ids=list(range(NCORES)), trace=_trace)
    _last_run_info.clear()
    _last_run_info.update(B=B, NE=NE, C=C, res=res)

    out = np.zeros((U, 64), np.float32)
    for k in range(NCORES):
        out_T = np.asarray(res.results[k]["out_T"]).astype(np.float32)
        cols = per_core[k]["col_of_slot"]
        out[k * U_PER:(k + 1) * U_PER] = out_T[:, cols].T
    return out


# revision 5
# speedup vs baseline: 1.1736x; 1.0094x over previous
"""Trainium2 Bass kernel for nn_HalfConv_876173328516 (GNN message passing).

Strategy (v2)
-------------
Host: sort edges by e_idx_u; core k owns u rows [k*6250, (k+1)*6250) so the 8
cores are fully independent. Edges are packed into 512-edge chunks whose u
values span < 48 slots. Per chunk the host emits:
  - a [128, 512] moving stream  x = [v_e(64) ; e_vals_e(16) ; onehot48_e(48)]
  - a [128, 128] stationary     lt = [W1v(64) ; W1e(16) ; Au_window(48)]
where Au = u @ g_w1[:64] (the per-u L1 contribution, precomputed on host), so
ONE K=128 N=512 matmul computes the full g-MLP layer-1 preactivation:
  z1[:, e] = W1v.T v_e + W1e.T e_e + Au[slot(e)].

Device (per core, per 512-edge chunk):
  L1   z1 = lt_c.T @ x_c                  (one N=512 matmul)
       h1 = relu(z1 + b1)                 (ACT -> SBUF bf16)
  L2   per 128-edge group: z2T = h1_g.T @ W2   (4x N=64 matmuls)
       h2T = max(z2T, -b2)                (DVE; relu(z+b) = max(z,-b)+b,
                                           the +deg*b2 lands in the flush)
  SUM  oh[e, slot] one-hot streamed from host (bf16)
       pT[64 feats, 48 slots] += h2T_g.T @ oh_g  (4x N=48, PSUM accumulate)
       xf[0:64, cols] = pT + b2*deg       (DVE add flush, bf16)
  f-MLP over compact slot columns: xf = [aggT ; uT], two matmuls + relus.

L1 for block m+1 is issued ahead of block m's tails so the PE never starves.

Host: out[u] = out_T[:, col_of_slot[u]].T per core.
"""

import numpy as np

U, V, E = 50000, 50000, 800000
NCORES = 8
U_PER = U // NCORES          # 6250
CHUNK = 512                  # edges per chunk
GP = 128                     # edges per matmul group
GROUPS = CHUNK // GP         # 4
WS = 48                      # slot window per chunk
F_DIM, G_DIM, H_DIM = 64, 64, 16


# ---------------------------------------------------------------- host side

def _preprocess(u, v, e_vals, e_idx_v, e_idx_u, g_w1, io_dtype_np):
    u = np.ascontiguousarray(np.asarray(u, np.float32))
    v = np.ascontiguousarray(np.asarray(v, np.float32))
    e_vals = np.ascontiguousarray(np.asarray(e_vals, np.float32))
    e_idx_u = np.asarray(e_idx_u).astype(np.int64)
    e_idx_v = np.asarray(e_idx_v).astype(np.int64)
    g_w1 = np.asarray(g_w1, np.float32)

    perm = np.argsort(e_idx_u, kind="stable")
    su = e_idx_u[perm]
    sv = e_idx_v[perm]
    se = e_vals[perm]

    core_lo = np.searchsorted(su, np.arange(NCORES) * U_PER, side="left")
    core_hi = np.searchsorted(su, (np.arange(NCORES) + 1) * U_PER, side="left")

    cores = []
    for k in range(NCORES):
        lo, hi = int(core_lo[k]), int(core_hi[k])
        su_l = (su[lo:hi] - k * U_PER).astype(np.int64)
        n = hi - lo
        starts, bases = [], []
        i = 0
        while i < n:
            base = int(su_l[i])
            j = min(i + CHUNK, n)
            j = min(j, int(np.searchsorted(su_l, base + WS, side="left")))
            if j < n:
                # step back to a u-boundary so no u straddles chunks
                j2 = int(np.searchsorted(su_l, su_l[j - 1], side="left"))
                if j2 > i and su_l[j - 1] == su_l[j]:
                    j = j2
            assert j > i, "u degree >= CHUNK unsupported"
            starts.append(i)
            bases.append(base)
            i = j
        starts.append(n)
        nchunks = len(bases)

        col_of_slot = np.full(U_PER, -1, np.int64)
        for c in range(nchunks):
            s0, s1 = starts[c], starts[c + 1]
            slots = np.unique(su_l[s0:s1])
            assert slots.max() - bases[c] < WS
            col_of_slot[slots] = WS * c + (slots - bases[c])
        uncovered = np.flatnonzero(col_of_slot < 0)
        cores.append(dict(lo=lo, hi=hi, su_l=su_l, sv=sv[lo:hi],
                          se=se[lo:hi], starts=starts, bases=bases,
                          nchunks=nchunks, col_of_slot=col_of_slot,
                          uncovered=uncovered))

    need = max(c["nchunks"] + (len(c["uncovered"]) + WS - 1) // WS
               for c in cores)
    B = need + (need % 2)            # chunks, in blocks of 2
    NE = B * CHUNK
    C = B * WS

    W1v = g_w1[F_DIM:F_DIM + G_DIM]              # [64, 128]
    W1e = g_w1[F_DIM + G_DIM:]                   # [16, 128]

    per_core = []
    for k in range(NCORES):
        ci = cores[k]
        su_l, starts, bases = ci["su_l"], ci["starts"], ci["bases"]
        nchunks = ci["nchunks"]
        n = ci["hi"] - ci["lo"]

        col_of_slot = ci["col_of_slot"].copy()
        unc = ci["uncovered"]
        if len(unc):
            cols = WS * nchunks + np.arange(len(unc))
            assert cols.max() < C
            col_of_slot[unc] = cols
        assert (col_of_slot >= 0).all()

        u_own = u[k * U_PER:(k + 1) * U_PER]     # [6250, 64]
        Au = (u_own @ g_w1[0:F_DIM]).astype(np.float32)   # [6250, 128]

        x_T = np.zeros((128, NE), np.float32)    # [vT ; eT ; onehot]
        lt = np.zeros((128, 128 * B), np.float32)
        oh4 = np.zeros((GP, WS * GROUPS * B), np.float32)
        deg = np.zeros(C, np.float32)
        if n:
            v_src = v[ci["sv"]].T                # [64, n]
            e_src = ci["se"].T                   # [16, n]
        for c in range(nchunks):
            s0, s1 = starts[c], starts[c + 1]
            m = s1 - s0
            base = bases[c]
            x_T[0:64, c * CHUNK:c * CHUNK + m] = v_src[:, s0:s1]
            x_T[64:80, c * CHUNK:c * CHUNK + m] = e_src[:, s0:s1]
            rel = su_l[s0:s1] - base
            j = np.arange(m)
            x_T[80 + rel, c * CHUNK + j] = 1.0
            lt[0:64, 128 * c:128 * (c + 1)] = W1v
            lt[64:80, 128 * c:128 * (c + 1)] = W1e
            hiu = min(base + WS, U_PER)
            lt[80:80 + hiu - base, 128 * c:128 * (c + 1)] = Au[base:hiu]
            oh4[j % GP, WS * GROUPS * c + WS * (j // GP) + rel] = 1.0
            deg[WS * c:WS * c + WS] = np.bincount(rel, minlength=WS)[:WS]

        u_T_compact = np.zeros((64, C), np.float32)
        u_T_compact[:, col_of_slot] = u_own.T

        per_core.append(dict(x_T=x_T, lt=lt, oh4=oh4,
                             u_T_compact=u_T_compact, deg=deg,
                             col_of_slot=col_of_slot))
    return per_core, B, NE, C


# ---------------------------------------------------------------- device side

def _build_program(B, NE, C, io_dtype_np, x_dtype_np, has_b2):
    import concourse.bacc as bacc
    import concourse.mybir as mybir
    import concourse.tile as tile

    FB = (C + 511) // 512               # f-MLP chunks
    MB = B // 2                         # blocks of 2 chunks
    md = mybir.dt.from_np(np.dtype(io_dtype_np))
    xd = mybir.dt.from_np(np.dtype(x_dtype_np))
    f32 = mybir.dt.float32
    Relu = mybir.ActivationFunctionType.Relu
    Alu = mybir.AluOpType

    nc = bacc.Bacc("TRN2", target_bir_lowering=False, debug=False,
                   num_devices=NCORES)

    # I/O
    x_T = nc.dram_tensor("x_T", [128, NE], xd, kind="ExternalInput")
    lt = nc.dram_tensor("lt", [128, 128 * B], md, kind="ExternalInput")
    oh4 = nc.dram_tensor("oh4", [GP, WS * GROUPS * B], xd,
                         kind="ExternalInput")
    if has_b2:
        corr = nc.dram_tensor("corr", [64, C], md, kind="ExternalInput")
    u_Tc = nc.dram_tensor("u_Tc", [64, C], md, kind="ExternalInput")
    w2 = nc.dram_tensor("w2", [128, 64], md, kind="ExternalInput")
    fw1 = nc.dram_tensor("fw1", [128, 128], md, kind="ExternalInput")
    fw2 = nc.dram_tensor("fw2", [128, 128], md, kind="ExternalInput")
    b1 = nc.dram_tensor("b1", [128, 1], f32, kind="ExternalInput")
    b2negm = nc.dram_tensor("b2negm", [GP, 64 * GROUPS], f32,
                            kind="ExternalInput")
    fb1 = nc.dram_tensor("fb1", [128, 1], f32, kind="ExternalInput")
    fb2 = nc.dram_tensor("fb2", [64, 1], f32, kind="ExternalInput")
    out_T = nc.dram_tensor("out_T", [64, C], md, kind="ExternalOutput")

    OHW = WS * GROUPS                   # one-hot cols per chunk

    with tile.TileContext(nc) as tc:
        with (
            tc.tile_pool(name="consts", bufs=1) as cp,
            tc.tile_pool(name="xf", bufs=1) as xfp,
            tc.tile_pool(name="xin", bufs=3) as xp,
            tc.tile_pool(name="ltin", bufs=3) as ltp,
            tc.tile_pool(name="ohin", bufs=3) as ohp,
            tc.tile_pool(name="wk3", bufs=3) as wp3,
            tc.tile_pool(name="wk2", bufs=2) as wp2,
            tc.tile_pool(name="pz1", bufs=2, space="PSUM") as pz1,
            tc.tile_pool(name="pz2", bufs=2, space="PSUM") as pz2,
            tc.tile_pool(name="ppT", bufs=1, space="PSUM") as ppT,
            tc.tile_pool(name="pf", bufs=1, space="PSUM") as pf,
        ):
            # input batches of 16 chunks (8 blocks), issued from the idle
            # GpSimd queue so descriptor writes overlap the const loads that
            # the Sync queue issues concurrently
            warm = cp.tile([1, 16], md)
            nc.gpsimd.dma_start(warm[:], lt[0:1, 0:16])

            BCH = 8                     # chunks per batch
            NBAT = (B + BCH - 1) // BCH

            def load_batch(bi):
                if bi >= NBAT:
                    return None
                wl = min(BCH * 128, 128 * B - bi * BCH * 128)
                lt_t = ltp.tile([128, BCH * 128], md, tag="lt")
                nc.gpsimd.dma_start(
                    lt_t[:, :wl], lt[:, bi * BCH * 128:bi * BCH * 128 + wl])
                w = min(BCH * CHUNK, NE - bi * BCH * CHUNK)
                xt = xp.tile([128, BCH * CHUNK], xd, tag="x1")
                nc.gpsimd.dma_start(
                    xt[:, :w], x_T[:, bi * BCH * CHUNK:bi * BCH * CHUNK + w])
                wo = min(BCH * OHW, OHW * B - bi * BCH * OHW)
                oh_t = ohp.tile([GP, BCH * OHW], xd, tag="oh4")
                nc.gpsimd.dma_start(
                    oh_t[:, :wo], oh4[:, bi * BCH * OHW:bi * BCH * OHW + wo])
                return xt, lt_t, oh_t

            bat0 = load_batch(0)

            # resident constants (on the Sync queue, parallel with batch 0)
            w2_s = cp.tile([128, 64], md)
            b1_s = cp.tile([128, 1], f32)
            b2negm_s = cp.tile([GP, 64 * GROUPS], f32)
            fw1_s = cp.tile([128, 128], md)
            fw2_s = cp.tile([128, 128], md)
            fb1_s = cp.tile([128, 1], f32)
            fb2_s = cp.tile([64, 1], f32)
            for dst, src in [(w2_s, w2), (b1_s, b1), (b2negm_s, b2negm),
                             (fw1_s, fw1), (fw2_s, fw2), (fb1_s, fb1),
                             (fb2_s, fb2)]:
                nc.sync.dma_start(dst[:], src[:])

            bats = {0: bat0, 1: load_batch(1), 2: load_batch(2)}

            # bulk constants after the first two batches
            xf = xfp.tile([128, C], md)
            nc.sync.dma_start(xf[64:128, :], u_Tc[:])
            if has_b2:
                corr_s = cp.tile([64, C], md)
                nc.sync.dma_start(corr_s[:], corr[:])

            def issue_l1(m):
                """L1 matmuls for block m into one [128, 1024] PSUM tile."""
                bat = bats[(2 * m) // BCH]
                z1 = pz1.tile([128, 2 * CHUNK], f32, tag="z1")
                for q in range(2):
                    off = (2 * m + q) % BCH
                    nc.tensor.matmul(
                        z1[:, q * CHUNK:(q + 1) * CHUNK],
                        lhsT=bat[1][:, off * 128:(off + 1) * 128],
                        rhs=bat[0][:, off * CHUNK:(off + 1) * CHUNK],
                        start=True, stop=True)
                return z1

            # f-MLP chunk emitter (interleaved into the main loop)
            f_done = [0]

            def emit_f(fc):
                w = min(512, C - 512 * fc)
                fsl = slice(512 * fc, 512 * fc + w)
                zf = pf.tile([128, 512], f32, tag="zf")
                nc.tensor.matmul(zf[:, :w], lhsT=fw1_s[:], rhs=xf[:, fsl],
                                 start=True, stop=True)
                hf = wp2.tile([128, 512], md, tag="hf")
                nc.scalar.activation(hf[:, :w], zf[:, :w], Relu,
                                     bias=fb1_s[:])
                nc.tensor.matmul(zf[:, :w], lhsT=fw2_s[:], rhs=hf[:, :w],
                                 start=True, stop=True)
                ot = wp2.tile([64, 512], md, tag="ot")
                nc.vector.tensor_scalar(ot[:, :w], zf[0:64, :w], fb2_s[:],
                                        0.0, op0=Alu.add, op1=Alu.max)
                nc.sync.dma_start(out_T[:, fsl], ot[:, :w])
                f_done[0] = fc + 1

            def issue_h1(z1):
                h1 = wp3.tile([128, 2 * CHUNK], md, tag="h1")
                nc.scalar.activation(h1[:], z1[:], Relu, bias=b1_s[:])
                return h1

            z1_cur = issue_l1(0)
            h1_cur = issue_h1(z1_cur)

            for m in range(MB):
                if m % (BCH // 2) == 0 and m > 0:
                    bi = m // (BCH // 2)
                    bats.pop(bi - 1, None)
                    bats[bi + 2] = load_batch(bi + 2)
                if m + 1 < MB:
                    z1_next = issue_l1(m + 1)
                    h1_next = issue_h1(z1_next)
                else:
                    z1_next = h1_next = None

                # tails for block m
                bat = bats[(2 * m) // BCH]
                h1 = h1_cur
                z2 = pz2.tile([128, 512], f32, tag="z2")
                for half in range(2):
                    for g in range(GROUPS):
                        j = half * GROUPS + g
                        nc.tensor.matmul(
                            z2[:, 64 * j:64 * (j + 1)],
                            lhsT=h1[:, GP * j:GP * (j + 1)],
                            rhs=w2_s[:], start=True, stop=True)
                # h2 padded to 128-col groups so the scatter LDWEIGHTS gets
                # FWL; output rows 64:128 of pT are garbage and never read
                pT = ppT.tile([GP, 2 * WS], f32, tag="pT")
                for q in range(2):
                    h2 = wp3.tile([GP, 4 * GP], md, tag="h2")
                    nc.vector.tensor_tensor(
                        h2[:].rearrange("p (g f) -> p g f", g=GROUPS)[:, :, 0:64],
                        z2[:, 256 * q:256 * (q + 1)]
                            .rearrange("p (g f) -> p g f", g=GROUPS),
                        b2negm_s[:].rearrange("p (g f) -> p g f", g=GROUPS),
                        op=Alu.max)
                    ohoff = ((2 * m + q) % BCH) * OHW
                    for g in range(GROUPS):
                        nc.tensor.matmul(
                            pT[:, WS * q:WS * (q + 1)],
                            lhsT=h2[:, GP * g:GP * (g + 1)],
                            rhs=bat[2][:, ohoff + WS * g:ohoff + WS * (g + 1)],
                            start=(g == 0), stop=(g == GROUPS - 1))
                if has_b2:
                    nc.vector.tensor_tensor(
                        xf[0:64, 2 * WS * m:2 * WS * (m + 1)], pT[0:64, :],
                        corr_s[:, 2 * WS * m:2 * WS * (m + 1)], op=Alu.add)
                else:
                    nc.vector.tensor_copy(
                        xf[0:64, 2 * WS * m:2 * WS * (m + 1)], pT[0:64, :])
                z1_cur, h1_cur = z1_next, h1_next
                while (f_done[0] + 1) * 512 <= (m + 1) * 2 * WS:
                    emit_f(f_done[0])

            for fc in range(f_done[0], FB):
                emit_f(fc)

    nc.compile()
    return nc


def _make_consts(g_w2, g_b1, g_b2, f_w1, f_b1, f_w2, f_b2, io_dtype_np):
    dt = io_dtype_np
    g_b2 = np.asarray(g_b2, np.float32)
    # f-MLP input is [aggT ; uT] (agg rows first), so permute f_w1 rows
    f_w1 = np.asarray(f_w1, np.float32)
    f_w1p = np.concatenate([f_w1[64:128], f_w1[0:64]], axis=0)
    return dict(
        w2=np.asarray(g_w2, np.float32).astype(dt),
        fw1=np.ascontiguousarray(f_w1p).astype(dt),
        fw2=np.concatenate([np.asarray(f_w2, np.float32),
                            np.zeros((128, 64), np.float32)],
                           axis=1).astype(dt),
        b1=np.asarray(g_b1, np.float32).reshape(128, 1),
        b2negm=np.ascontiguousarray(
            np.tile(-g_b2[None, :], (GP, GROUPS))).astype(np.float32),
        fb1=np.asarray(f_b1, np.float32).reshape(128, 1),
        fb2=np.asarray(f_b2, np.float32).reshape(64, 1),
    )


_last_run_info = {}


def kernel(u, v, e_vals, e_idx_v, e_idx_u, g_w1, g_b1, g_w2, g_b2,
           f_w1, f_b1, f_w2, f_b2, _trace=False):
    import ml_dtypes
    from concourse import bass_utils

    io_dtype_np = ml_dtypes.bfloat16
    x_dtype_np = ml_dtypes.float8_e3m4

    g_b2f = np.asarray(g_b2, np.float32)
    has_b2 = bool(np.any(g_b2f))

    per_core, B, NE, C = _preprocess(u, v, e_vals, e_idx_v, e_idx_u,
                                     g_w1, io_dtype_np)
    consts = _make_consts(g_w2, g_b1, g_b2, f_w1, f_b1, f_w2, f_b2,
                          io_dtype_np)
    nc = _build_program(B, NE, C, io_dtype_np, x_dtype_np, has_b2)

    in_maps = []
    for pc in per_core:
        m = dict(
            x_T=np.clip(pc["x_T"], -15.0, 15.0).astype(x_dtype_np),
            lt=pc["lt"].astype(io_dtype_np),
            oh4=pc["oh4"].astype(x_dtype_np),
            u_Tc=pc["u_T_compact"].astype(io_dtype_np),
            **consts,
        )
        if has_b2:
            m["corr"] = (g_b2f[:, None] * pc["deg"][None, :]) \
                .astype(io_dtype_np)
        in_maps.append(m)

    res = bass_utils.run_bass_kernel_spmd(
        nc, in_maps, core_ids=list(range(NCORES)), trace=_trace)
    _last_run_info.clear()
    _last_run_info.update(B=B, NE=NE, C=C, res=res)

    out = np.zeros((U, 64), np.float32)
    for k in range(NCORES):
        out_T = np.asarray(res.results[k]["out_T"]).astype(np.float32)
        cols = per_core[k]["col_of_slot"]
        out[k * U_PER:(k + 1) * U_PER] = out_T[:, cols].T
    return out
